# revision 10
# baseline (speedup 1.0000x reference)
"""Trainium2 Bass kernel for nn_AttentionLayer (sparse_attention).

Reference computation (B=4, N=2048, C=256, H=8, HD=32):
    qkv = x @ qkv_w.T; q,k,v = split(qkv); heads
    scores = q k^T / sqrt(HD) + adj          [B,H,N,N]
    out    = softmax(scores) @ v             -> merge heads [B,N,C]
    result = (out*0.1 + x) @ out_w.T + out_b
(The pos_proj(adj) value in the reference is dead code; x0 is unused.)

Sharding: 8 cores = (batch b, query-half).  Core c handles batch c//2 and
query rows [(c%2)*1024, (c%2+1)*1024).  Each core computes K/V for its
whole batch locally (no collectives); the host rolls the key axis so the
core's own query rows are rows 0..1023 of its x input, and rolls adj
columns the same way (softmax is key-permutation invariant).

fp8 / DoubleRow design (this version):
  * q,k are stored as fp8e4 in [P, 2, len] "zero-slot" layout (slot 1 is
    zeros), so score matmuls run in DoubleRow perf mode at 0.5 cyc/col —
    the 2x PE speedup needs no hd repartitioning because matmul cost
    depends only on the moving free size.
  * adj is pre-added into PSUM by the PE itself: adj*16 is split on the
    host into c (exact multiples of 8 in fp8) + r (|r|<=4), shipped as an
    interleaved [P,16,2,NQ] fp8 tensor, and accumulated through a
    DoubleRow identity matmul (slot0=c, slot1=r -> full bf16-grade
    precision at 0.5 cyc/col).  This removes ALL per-tile DVE/Pool
    combine work (the old kernel spent ~110us of DVE on adj adds).
  * ACT exps PSUM scores straight to fp8 E tiles (scale=1/16 folds the
    q/k fp8 scaling; bias=-2ln2 rescales E by 1/4 so exp(max) stays
    under the fp8e4 normal max of 240 — the softmax ratio is invariant).
  * attention@V runs DoubleRow over KEY-TILE PAIRS: E tiles are
    [P, 2(kt-parity), NQ] fp8 and v is fp8 [P,16,H,34] so one call
    contracts 256 keys -> attnv drops 4x vs the bf16 version.
  * out = E @ [v | 10]: row 32 of po gives 10*sum(E) (softmax denominator
    with the 0.1 output scale folded).  Reciprocal runs on a DMA-spread
    [64,32] layout; po is copied PSUM->SBUF early so the PSUM bank frees
    for the next head pair.
Engine budget (est): PE ~86us, ACT ~131us (exp bound), DVE ~40us.
"""

import sys

for _p in ("/opt/trn_rl_repo", "/root/.axon_site/_ro/trn_rl_repo"):
    if _p not in sys.path:
        sys.path.insert(0, _p)

import ml_dtypes
import numpy as np

import concourse.mybir as mybir
from concourse import bacc
from concourse.bass import ds, ts
from concourse.tile import TileContext

B, N, C, H = 4, 2048, 256, 8
HD = C // H          # 32
NQ = N // 2          # 1024 query rows per core
SCALE = 1.0 / np.sqrt(HD)
FP32 = mybir.dt.float32
BF16 = mybir.dt.bfloat16
F8 = mybir.dt.float8e4
P = 128
BF16NP = ml_dtypes.bfloat16
F8NP = ml_dtypes.float8_e4m3

QS, KS = 8.0, 2.0            # fp8 pre-scales folded into host-side q/k weights
SSC = 1.0 / (QS * KS)        # exp() scale param undoing them
EBIAS = float(-2.0 * np.log(2.0))   # E *= 1/4: keeps exp under fp8e4 max
ADJ_C_STEP = 8.0             # c-part quantization step of adj*16
VW = 64                      # vF8 head stride (DoubleRow needs 64-col stationary tiles)

_CACHED = {}

PAIRS = ((0, 2), (1, 3), (4, 6), (5, 7))
DR = mybir.MatmulPerfMode.DoubleRow


def build_kernel(repeat=1):
    nc = bacc.Bacc("TRN2", target_bir_lowering=False)
    xt_ext = nc.declare_dram_parameter("xt", [C, N], BF16, isOutput=False)
    adjcr_ext = nc.declare_dram_parameter("adjcr", [P, 16, 2, NQ], F8, isOutput=False)
    i2_ext = nc.declare_dram_parameter("i2", [P, 2, P], F8, isOutput=False)
    wt_ext = nc.declare_dram_parameter("qkv_wt", [C, 3 * C], BF16, isOutput=False)
    owt_ext = nc.declare_dram_parameter("out_wt", [C, C], BF16, isOutput=False)
    outb_ext = nc.declare_dram_parameter("out_b", [P, C], FP32, isOutput=False)
    out_ext = nc.declare_dram_parameter("out", [NQ, C], FP32, isOutput=True)

    with TileContext(nc) as tc:
        with (
            tc.tile_pool(name="const", bufs=1) as constp,
            tc.tile_pool(name="persist", bufs=1) as persist,
            tc.tile_pool(name="work", bufs=2) as work,
            tc.tile_pool(name="sp_pool", bufs=2, space="PSUM") as spp,
            tc.tile_pool(name="po_pool", bufs=2, space="PSUM") as pop,
        ):
            outb_bc = constp.tile([P, C], FP32)
            nc.sync.dma_start(outb_bc[:], outb_ext[:, :])
            i2t = constp.tile([P, 2, P], F8)
            nc.sync.dma_start(i2t[:], i2_ext[:, :, :])
            ebias = constp.tile([P, 1], FP32)
            nc.vector.memset(ebias[:], EBIAS)
            for _ in range(repeat):
                _body(nc, tc, persist, work, spp, pop, outb_bc, i2t, ebias,
                      xt_ext, adjcr_ext, wt_ext, owt_ext, out_ext)

    nc.compile()
    return nc


def _body(nc, tc, persist, work, spp, pop, outb_bc, i2t, ebias,
          xt_ext, adjcr_ext, wt_ext, owt_ext, out_ext):
    AF = mybir.ActivationFunctionType
    ALU = mybir.AluOpType

    # ---------------- persistent SBUF tensors ----------------
    xT = [persist.tile([P, N], BF16, tag=f"xT{i}", name=f"xT{i}") for i in range(2)]
    wT = [persist.tile([P, 3 * C], BF16, tag=f"wT{i}", name=f"wT{i}") for i in range(2)]
    owT = [persist.tile([P, C], BF16, tag=f"owT{i}", name=f"owT{i}") for i in range(2)]
    # fp8 head stacks: head h lives in stack h//3 at rows 32*(h%3);
    # layout [P, 2, len] with slot 1 kept zero (DoubleRow zero-slot trick)
    kF8 = [persist.tile([P, 2, N], F8, tag=f"kF8{i}", name=f"kF8{i}")
           for i in range(3)]
    qF8 = [persist.tile([P, 2, NQ], F8, tag=f"qF8{i}", name=f"qF8{i}")
           for i in range(3)]
    vF8 = persist.tile([P, 16, H, VW], F8, tag="vF8")
    adjCR = persist.tile([P, 16, 2, NQ], F8, tag="adjCR")
    attT = [persist.tile([P, NQ], BF16, tag=f"attT{i}", name=f"attT{i}")
            for i in range(2)]

    # ---------------- loads (already transposed/scaled on host) -----------
    for j in range(2):
        for hseg in range(2):
            nc.sync.dma_start(xT[j][:, ds(hseg * NQ, NQ)],
                              xt_ext[ds(j * P, P), ds(hseg * NQ, NQ)])
        nc.sync.dma_start(wT[j][:], wt_ext[ds(j * P, P), :])
        nc.sync.dma_start(owT[j][:], owt_ext[ds(j * P, P), :])
    for i in range(16):
        nc.gpsimd.dma_start(adjCR[:, i, :, :], adjcr_ext[:, i, :, :])

    # zero-slot prep on the otherwise idle Pool engine (slot1 of q/k must
    # be actual zeros, not uninitialized SBUF)
    for t in (*kF8, *qF8):
        nc.gpsimd.memset(t[:], 0.0)

    # ---------------- QKV projections (bf16 PE, fp8 outputs) --------------
    def proj_stack(dst, off, nchs):
        for j in range(3):
            w0 = 96 * j
            for nch in range(nchs):
                pp = spp.tile([P, NQ], FP32, tag="sp", name="pp")[:, :512]
                for (cb, cw) in ((0, 64), (64, 32)) if j < 2 else ((0, 64),):
                    for cc in range(2):
                        nc.tensor.matmul(pp[ds(cb, cw), :],
                                         wT[cc][:, ds(off + w0 + cb, cw)],
                                         xT[cc][:, ts(nch, 512)],
                                         start=(cc == 0), stop=(cc == 1))
                rows = 96 if j < 2 else 64
                nc.vector.tensor_copy(dst[j][ds(0, rows), 0, ts(nch, 512)],
                                      pp[ds(0, rows), :])

    proj_stack(qF8, 0, 2)
    proj_stack(kF8, C, 4)
    # v: [key_tile, head, hd] fp8 with ones column scaled by 10 (folds 0.1);
    # cols 33..63 are zero padding (DoubleRow wants 64-col stationary tiles)
    nc.gpsimd.memset(vF8[:], 0.0)
    nc.vector.memset(vF8[:, :, :, HD], 10.0)
    for kt in range(16):
        pv = spp.tile([P, NQ], FP32, tag="sp", name="pv")[:, :512]
        for cc in range(2):
            nc.tensor.matmul(pv[:, :C], xT[cc][:, ts(kt, P)],
                             wT[cc][:, ds(2 * C, C)],
                             start=(cc == 0), stop=(cc == 1))
        nc.vector.tensor_copy(
            vF8[:, kt, :, 0:HD],
            pv[:, :C].rearrange("p (h d) -> p h d", h=H))

    # ---------------- attention: one head at a time ----------------
    # (DoubleRow matmuls require output tile_position col 0, so each head's
    # attnv accumulates into rows 0..63 of its own rotating po tile)
    for h in range(H):
        sh, rh = h // 3, 32 * (h % 3)
        po = pop.tile([P, NQ], FP32, tag="po", name="po", bufs=2)
        for tp in range(8):
            E2 = work.tile([P, 2, NQ], F8, tag="E2", name="E2", bufs=4)
            for half in range(2):
                kt = 2 * tp + half
                sp = spp.tile([P, NQ], FP32, tag="sp", name="sp")
                for c in range(4):
                    # adj preload: DoubleRow identity matmul, slot0=c slot1=r
                    nc.tensor.matmul(sp[:, ds(256 * c, 256)],
                                     i2t[:, :, :],
                                     adjCR[:, kt, :, ds(256 * c, 256)],
                                     start=True, stop=False, perf_mode=DR,
                                     skip_group_check=True)
                    nc.tensor.matmul(sp[:, ds(256 * c, 256)],
                                     kF8[sh][ds(rh, HD), :, ts(kt, P)],
                                     qF8[sh][ds(rh, HD), :, ds(256 * c, 256)],
                                     start=False, stop=True, perf_mode=DR,
                                     skip_group_check=True)
                nc.scalar.activation(E2[:, half, :], sp[:], AF.Exp,
                                     bias=ebias[:], scale=SSC)
            # attnv: DoubleRow over the kt pair (contracts 256 keys/call)
            for c in range(4):
                nc.tensor.matmul(po[ds(0, 64), ds(256 * c, 256)],
                                 vF8[:, ds(2 * tp, 2), h, ds(0, 64)],
                                 E2[:, :, ds(256 * c, 256)],
                                 start=(tp == 0), stop=(tp == 7), perf_mode=DR,
                                 skip_group_check=True)

        # ---- free PSUM fast: copy po to SBUF, then normalize from there ----
        por = work.tile([P, NQ], FP32, tag="por", name="por")
        nc.vector.tensor_copy(por[:], po[:])
        # reciprocal on a DMA-spread [32,32] layout (DVE recip cost scales
        # with free size; the spread form is ~16x cheaper than [1,1024])
        dsp = work.tile([32, 32], FP32, tag="dsp", name="dsp")
        nc.sync.dma_start(dsp[:],
                          por[ds(HD, 1), :].rearrange("o (p j) -> o p j", p=32))
        rc = work.tile([32, 32], FP32, tag="rc", name="rc")
        nc.vector.reciprocal(rc[:], dsp[:])
        rr = work.tile([1, NQ], FP32, tag="rr", name="rr")
        nc.sync.dma_start(rr[:].rearrange("o (p j) -> o p j", p=32), rc[:])
        bc = work.tile([HD, NQ], FP32, tag="bc", name="bc")
        nc.sync.dma_start(bc[:], rr[ds(0, 1), None, :].to_broadcast((1, HD, NQ)))
        home, chunk = 32 * (h % 4), h // 4
        nc.vector.tensor_tensor(attT[chunk][ds(home, HD), :],
                                por[ds(0, HD), :], bc[:], ALU.mult)

    # ---------------- residual + out_proj ----------------
    for cc in range(2):
        nc.vector.tensor_tensor(attT[cc][:], attT[cc][:], xT[cc][:, 0:NQ],
                                ALU.add)
    for rt in range(8):
        pf = spp.tile([P, NQ], FP32, tag="sp", name="pf")[:, :C]
        for cc in range(2):
            nc.tensor.matmul(pf[:, :C], attT[cc][:, ts(rt, P)], owT[cc][:],
                             start=(cc == 0), stop=(cc == 1))
        osb = work.tile([P, C], FP32, tag="osb", name="osb")
        nc.vector.tensor_tensor(osb[:], pf[:, :C], outb_bc[:], ALU.add)
        nc.sync.dma_start(out_ext[ds(rt * P, P), :], osb[:])


def _run(nc, in_maps):
    from concourse.bass_utils import run_bass_kernel_spmd
    res = run_bass_kernel_spmd(nc, in_maps, core_ids=list(range(8)))
    return res.results


def make_in_maps(x, adj, qkv_w, out_w, out_b):
    x = np.asarray(x, np.float32)
    adj = np.asarray(adj, np.float32)
    w = np.asarray(qkv_w, np.float32).copy()
    w[:C] *= SCALE * QS                  # fold 1/sqrt(HD) + fp8 q scale
    w[C:2 * C] *= KS                     # fp8 k scale
    wt = np.ascontiguousarray(w.T).astype(BF16NP)
    owt = np.ascontiguousarray(np.asarray(out_w, np.float32).T).astype(BF16NP)
    outb = np.ascontiguousarray(
        np.broadcast_to(np.asarray(out_b, np.float32), (P, C)))
    i2 = np.zeros((P, 2, P), F8NP)
    for p in range(P):
        i2[p, :, p] = F8NP(1.0)
    in_maps = []
    for c in range(8):
        b, half = divmod(c, 2)
        xb = np.roll(x[b], -half * NQ, axis=0)
        xt = np.ascontiguousarray(xb.T).astype(BF16NP)          # [C, N]
        aj = np.roll(adj[half * NQ:(half + 1) * NQ, :], -half * NQ, axis=1)
        a16 = aj.T * (QS * KS)                                  # [N, NQ]
        ac = np.round(a16 / ADJ_C_STEP) * ADJ_C_STEP
        ar = a16 - ac
        # [N, NQ] -> [16, P, NQ] -> [P, 16, NQ]; stack (c, r) on a new axis
        acT = ac.reshape(16, P, NQ).transpose(1, 0, 2)
        arT = ar.reshape(16, P, NQ).transpose(1, 0, 2)
        adjcr = np.ascontiguousarray(
            np.stack([acT, arT], axis=2)).astype(F8NP)          # [P,16,2,NQ]
        in_maps.append({
            "xt": xt, "adjcr": adjcr, "i2": i2,
            "qkv_wt": wt, "out_wt": owt, "out_b": outb,
        })
    return in_maps


def kernel(x, x0, adj, qkv_w, out_w, out_b, pos_w, pos_b):
    """Full-input, full-output entry point.  x0/pos_w/pos_b are dead in the
    reference computation and are ignored."""
    if "nc" not in _CACHED:
        _CACHED["nc"] = build_kernel(repeat=1)
    nc = _CACHED["nc"]
    in_maps = make_in_maps(x, adj, qkv_w, out_w, out_b)
    results = _run(nc, in_maps)
    out = np.empty((B, N, C), np.float32)
    for c in range(8):
        b, half = divmod(c, 2)
        out[b, half * NQ:(half + 1) * NQ, :] = results[c]["out"]
    return out
